# revision 1
# baseline (speedup 1.0000x reference)
"""Trainium2 Bass kernel for nn_AttentionLayer (B=4, S=4096, D=128, fp32).

Sharding: batch (4) x query-half (2) across 8 NeuronCores. Each core computes
single-head attention for one batch element over a 2048-query slice with full
4096-key context.

Per-core dataflow (all on-chip after the x^T load; all matmuls f32r):
  K^T[e,t] = WkT.T @ x^T             (PE, N=512)
  Q^T[e,s] = WqT.T @ x^T[:,qcols]    (PE, N=512)
  [V[t,e] | alpha[t]] = x^T-chunk.T @ [WvT | wtl]   (PE, N=130)
  scoresT[t-chunk, s] = K^T-chunk.T @ Q^T           (PE -> PSUM)
  expT = exp(scale*scoresT + alpha)  (ACT, PSUM -> SBUF, pipelined 2 ahead)
  outT[e,s] += V-chunk.T @ expT      (PE, PSUM accumulate)
  denom: every 4th chunk ones.T @ expT on PE (PSUM, replicated rows); the
         rest accumulate on DVE, folded in by one plain-f32 ones-matmul
  outT = outT * recip(denom) + bv    (DVE), DMA'd out as [e, s]; the host
         does the final [e,s] -> [s,e] layout flip while gathering cores.

Bias algebra: the query-side bias terms (q0.bk, bq.bk) are constant over
keys and cancel in softmax; the key-side term bq.k0[t] is folded into the
exp bias via alpha = x^T.T @ (scale * Wk.T @ bq), computed as two extra
columns of the V projection. bk drops out entirely; bv is added at the end
(attention weights sum to 1 after normalization).
"""

import sys

import numpy as np

for _p in ("/opt/trn_rl_repo", "/opt/pypackages"):
    if _p not in sys.path:
        sys.path.append(_p)

B, S, D = 4, 4096, 128
N_CORES = 8
SQ = S // 2  # queries per core
SCALE = 1.0 / float(np.sqrt(D))


def build_attention_bass(s=S, sq=SQ, sw=1024):
    """Build the single-core SPMD Bass program.

    s: key/context length; sq: queries handled by the core; sw: query-pass
    width (PSUM budget: 2*sw*4B of score buffers + sw*4B out + sw*4B denom
    per partition must fit 16KB -> sw=1024 uses exactly 8 banks).
    """
    import concourse.bass as bass
    import concourse.mybir as mybir
    import concourse.tile as tile
    from concourse import bacc
    from contextlib import ExitStack

    f32 = mybir.dt.float32
    f32r = mybir.dt.float32r
    FT = mybir.ActivationFunctionType

    tch = s // 128          # key chunks
    n_pass = sq // sw       # query passes
    nw = min(512, sw)       # matmul N width
    jn = sw // nw           # matmuls per pass-width

    def chunks(total, w=512):
        for st in range(0, total, w):
            yield st, min(w, total - st)

    nc = bacc.Bacc("TRN2", target_bir_lowering=False, debug=False)

    xT = nc.dram_tensor("xT", [D, s], f32r, kind="ExternalInput").ap()
    xTq = nc.dram_tensor("xTq", [D, sq], f32r, kind="ExternalInput").ap()
    wqT = nc.dram_tensor("wqT", [D, D], f32r, kind="ExternalInput").ap()
    wkT = nc.dram_tensor("wkT", [D, D], f32r, kind="ExternalInput").ap()
    wvT = nc.dram_tensor("wvT", [D, D + 2], f32r, kind="ExternalInput").ap()
    bv = nc.dram_tensor("bv", [D, 1], f32, kind="ExternalInput").ap()
    out_d = nc.dram_tensor("out", [D, sq], f32, kind="ExternalOutput").ap()

    with tile.TileContext(nc) as tc, ExitStack() as ctx:
        const = ctx.enter_context(tc.tile_pool(name="const", bufs=1))
        big = ctx.enter_context(tc.tile_pool(name="big", bufs=1))
        exp_pool = ctx.enter_context(tc.tile_pool(name="exp", bufs=5))
        epi = ctx.enter_context(tc.tile_pool(name="epi", bufs=2))
        outp = ctx.enter_context(tc.tile_pool(name="outp", bufs=3))

        # ---- constants / weights
        wq_sb = const.tile([D, D], f32r, tag="wq")
        wk_sb = const.tile([D, D], f32r, tag="wk")
        wv_sb = const.tile([D, D + 2], f32r, tag="wv")
        bv_sb = const.tile([D, 1], f32, tag="bv")
        ones_sb = const.tile([128, 128], f32r, tag="ones")
        ones_f32 = const.tile([128, 128], f32, tag="ones32")
        # ---- input DMAs, ordered so pass-0-critical data lands first:
        # wk/wq/wv, x^T cols 0:512 (first K chunk), query cols 0:1024
        # (pass-0 scores), then the rest round-robins across queues
        xT_sb = big.tile([D, s], f32r, tag="xT")
        xTq_sb = big.tile([D, sq], f32r, tag="xTq")
        nc.sync.dma_start(wk_sb[:], wkT)
        nc.sync.dma_start(wq_sb[:], wqT)
        nc.sync.dma_start(wv_sb[:], wvT)
        xt_chunks = list(chunks(s, 256))
        xtq_chunks = list(chunks(sq, 256))
        early_xt, late_xt = xt_chunks[:2], xt_chunks[2:]
        early_q, late_q = xtq_chunks[:4], xtq_chunks[4:]
        for st, w in early_xt:
            nc.sync.dma_start(xT_sb[:, st:st + w], xT[:, st:st + w])
        for st, w in early_q:
            nc.sync.dma_start(xTq_sb[:, st:st + w], xTq[:, st:st + w])
        for st, w in late_xt:
            nc.sync.dma_start(xT_sb[:, st:st + w], xT[:, st:st + w])
        for st, w in late_q:
            nc.sync.dma_start(xTq_sb[:, st:st + w], xTq[:, st:st + w])
        nc.sync.dma_start(bv_sb[:], bv)
        # f32r memset is illegal; memset an f32 ones tile, then round it to
        # f32r on ACT so the in-loop f32r denominator matmuls accept it
        nc.vector.memset(ones_f32[:], 1.0)
        nc.scalar.activation(ones_sb[:], ones_f32[:],
                             FT.Identity, bias=1.0, scale=0.0)

        kt_sb = big.tile([D, s], f32r, tag="kt")
        qt_sb = big.tile([D, sq], f32r, tag="qt")
        # per key-chunk: cols [130c, 130c+128) = V chunk, col 130c+128 = alpha
        v_sb = big.tile([128, (D + 2) * tch], f32r, tag="v")
        accdv = ctx.enter_context(tc.tile_pool(name="accdv", bufs=2))

        with tc.tile_pool(name="scps", bufs=2, space="PSUM") as scps:
            # projection pools live only until the pass loop starts; their 4
            # banks are then handed to the accumulator pool (8-bank budget)
            qkv_ctx = ExitStack()
            qkps = qkv_ctx.enter_context(
                tc.tile_pool(name="qkps", bufs=4, space="PSUM"))
            vps = qkps

            kqw = min(256, max(130, s))  # match the 256-col x DMA chunks

            def emit_k(j):
                st, w = j * kqw, min(kqw, s - j * kqw)
                kp = qkps.tile([128, kqw], f32, tag="kp")
                nc.tensor.matmul(kp[:, :w], wk_sb[:], xT_sb[:, st:st + w])
                nc.vector.tensor_copy(kt_sb[:, st:st + w], kp[:, :w])

            def emit_q(j):
                st, w = j * kqw, min(kqw, sq - j * kqw)
                qp = qkps.tile([128, kqw], f32, tag="kp")
                nc.tensor.matmul(qp[:, :w], wq_sb[:], xTq_sb[:, st:st + w])
                nc.vector.tensor_copy(qt_sb[:, st:st + w], qp[:, :w])

            def emit_scores_exp(p, c):
                """scores chunk c of pass p -> PSUM, exp -> SBUF (f32r)."""
                sc = scps.tile([128, sw], f32, tag="sc")
                kc = kt_sb[:, c * 128:(c + 1) * 128]
                for j in range(jn):
                    nc.tensor.matmul(
                        sc[:, j * nw:(j + 1) * nw], kc,
                        qt_sb[:, p * sw + j * nw: p * sw + (j + 1) * nw])
                et = exp_pool.tile([128, sw], f32r, tag="et")
                ac = c * (D + 2) + D
                nc.scalar.activation(et[:], sc[:], FT.Exp,
                                     bias=v_sb[:, ac:ac + 1].bitcast(f32),
                                     scale=SCALE)
                return et

            # K/Q needed by pass-0 scores first, then pre-emit 2 score chunks
            emit_k(0)
            nk, nq = (s + kqw - 1) // kqw, (sq + kqw - 1) // kqw
            q_pass0 = max(1, min(nq, (sw + kqw - 1) // kqw))
            for j in range(q_pass0):
                emit_q(j)
            def emit_v_alpha(c):
                # one N=130 matmul: cols 0..127 -> V chunk, col 128 -> alpha
                vp = vps.tile([128, D + 2], f32, tag="kp")
                xc = xT_sb[:, c * 128:(c + 1) * 128]
                nc.tensor.matmul(vp[:], xc, wv_sb[:])
                nc.vector.tensor_copy(
                    v_sb[:, c * (D + 2):(c + 1) * (D + 2)], vp[:])

            npre = min(2, tch)
            for c in range(npre):
                emit_v_alpha(c)
            pre = [emit_scores_exp(0, c) for c in range(npre)]
            # rest of the projections (fills PE while ACT runs the first exps)
            for j in range(1, nk):
                emit_k(j)
            for j in range(q_pass0, nq):
                emit_q(j)
            for c in range(npre, tch):
                emit_v_alpha(c)

            qkv_ctx.close()
            acc_ctx = ExitStack()
            accps = acc_ctx.enter_context(
                tc.tile_pool(name="accps", bufs=1, space="PSUM"))

            # ---- attention passes (scores/exp pipelined 2 chunks ahead;
            # denominator chunk-accumulated on DVE, partition-reduced by a
            # single f32 ones-matmul per pass)
            sched = [(pp, cc) for pp in range(n_pass) for cc in range(tch)]
            cursor = [len(pre)]
            ets = pre
            for p in range(n_pass):
                acc_o = accps.tile([128, sw], f32, tag="acco")
                acc_d = accps.tile([128, sw], f32, tag="accd")
                acc_dv = accdv.tile([128, sw], f32r, tag="accdv")
                first_dv = True
                for c in range(tch):
                    et = ets.pop(0)
                    if cursor[0] < len(sched):
                        pp, cc = sched[cursor[0]]
                        cursor[0] += 1
                        ets.append(emit_scores_exp(pp, cc))
                    vc = v_sb[:, c * (D + 2):c * (D + 2) + D]
                    for j in range(jn):
                        nc.tensor.matmul(acc_o[:, j * nw:(j + 1) * nw], vc,
                                         et[:, j * nw:(j + 1) * nw],
                                         start=(c == 0), stop=(c == tch - 1))
                    # denominator: every 4th chunk on PE (f32r ones-matmul),
                    # the rest chunk-accumulated on DVE
                    if c % 4 == 0 or c == tch - 1:
                        # last chunk stays on PE so the reciprocal chain
                        # does not wait for a trailing DVE add
                        for j in range(jn):
                            nc.tensor.matmul(acc_d[:, j * nw:(j + 1) * nw],
                                             ones_sb[:],
                                             et[:, j * nw:(j + 1) * nw],
                                             start=(c == 0), stop=False)
                    elif first_dv:
                        nc.vector.tensor_copy(acc_dv[:], et[:])
                        first_dv = False
                    else:
                        nc.vector.tensor_add(acc_dv[:], acc_dv[:], et[:])
                # deepen the pipeline across the pass boundary so the PE has
                # score work while the epilogue chain drains on DVE
                while cursor[0] < len(sched) and len(ets) < 4:
                    pp, cc = sched[cursor[0]]
                    cursor[0] += 1
                    ets.append(emit_scores_exp(pp, cc))
                # fold the DVE partial sums in: ones.T @ acc_dv
                assert not first_dv, "pass had no DVE denominator chunks"
                for j in range(jn):
                    nc.tensor.matmul(acc_d[:, j * nw:(j + 1) * nw],
                                     ones_sb[:],
                                     acc_dv[:, j * nw:(j + 1) * nw],
                                     start=False, stop=True)
                # normalize + bias, then DMA the [e, s] block straight out
                # (host does the final layout transpose); finer blocks on the
                # last pass so the tail output DMAs start earlier
                bw = min(nw, 256) if p == n_pass - 1 else nw
                for b0 in range(0, sw, bw):
                    recip = epi.tile([128, bw], f32, tag="recip")
                    nc.vector.reciprocal_approx_fast(
                        recip[:], acc_d[:, b0:b0 + bw])
                    norm = epi.tile([128, bw], f32, tag="norm")
                    nc.vector.tensor_mul(norm[:], acc_o[:, b0:b0 + bw],
                                         recip[:])
                    norm2 = outp.tile([128, bw], f32, tag="norm2")
                    nc.vector.tensor_scalar_add(norm2[:], norm[:], bv_sb[:])
                    c0 = p * sw + b0
                    nc.sync.dma_start(out_d[:, c0:c0 + bw], norm2[:])
            acc_ctx.close()
    nc.compile()
    return nc


def make_in_maps(x, Wq, bq, Wk, Wv, bv, s=S, sq=SQ, n_cores=N_CORES):
    """Per-core input dict list. Core c -> batch c//(cores per batch),
    query slice (c % per_b) * sq."""
    x = np.asarray(x, np.float32)
    nb = x.shape[0]
    per_b = n_cores // nb
    wq_t = np.ascontiguousarray(np.asarray(Wq, np.float32).T)
    wk_t = np.ascontiguousarray(np.asarray(Wk, np.float32).T)
    wv_t = np.ascontiguousarray(np.asarray(Wv, np.float32).T)
    wtl = (SCALE * (wk_t @ np.asarray(bq, np.float32))).reshape(D, 1)
    wv_aug = np.concatenate([wv_t, wtl, wtl], axis=1)
    bvc = np.asarray(bv, np.float32).reshape(D, 1)
    maps = []
    for c in range(n_cores):
        b, h = c // per_b, c % per_b
        xt = np.ascontiguousarray(x[b].T)
        maps.append({
            "xT": xt,
            "xTq": np.ascontiguousarray(xt[:, h * sq:(h + 1) * sq]),
            "wqT": wq_t, "wkT": wk_t,
            "wvT": np.ascontiguousarray(wv_aug, dtype=np.float32),
            "bv": np.ascontiguousarray(bvc, dtype=np.float32),
        })
    return maps


_NC_CACHE = {}


def _get_nc():
    if "nc" not in _NC_CACHE:
        _NC_CACHE["nc"] = build_attention_bass()
    return _NC_CACHE["nc"]


def run_on_hw(inputs, trace=False, **kw):
    from concourse.bass_utils import run_bass_kernel_spmd
    nc = _get_nc()
    maps = make_in_maps(inputs["x"], inputs["Wq"], inputs["bq"], inputs["Wk"],
                        inputs["Wv"], inputs["bv"])
    res = run_bass_kernel_spmd(nc, maps, core_ids=list(range(N_CORES)),
                               trace=trace, **kw)
    nb = np.asarray(inputs["x"]).shape[0]
    per_b = N_CORES // nb
    out = np.empty((nb, S * D), np.float32)
    for c in range(N_CORES):
        b, h = c // per_b, c % per_b
        # device returns out^T [D, SQ]; final layout flip happens here
        out[b, h * SQ * D:(h + 1) * SQ * D] = \
            np.asarray(res.results[c]["out"]).T.reshape(-1)
    return out, res


def kernel(**inputs):
    out, _ = run_on_hw(inputs, trace=False)
    return out



# revision 7
# speedup vs baseline: 1.0813x; 1.0813x over previous
"""Trainium2 Bass kernel for nn_AttentionLayer (B=4, S=4096, D=128, fp32).

Sharding: batch (4) x query-half (2) across 8 NeuronCores. Each core computes
single-head attention for one batch element over a 2048-query slice with full
4096-key context.

Math (host-side algebra):
  scores[s,t] = q[s]@k[t]/sqrt(d) = x[s] G x[t]^T + alpha[t] + const(s),
  G = Wq^T Wk / sqrt(d), alpha[t] = x[t]@(Wk^T bq)/sqrt(d).
  const(s) cancels in softmax; bk drops entirely; bv is added at the end.
  The Q projection disappears: k''[t] = x[t] G^T is the only "key" tensor and
  raw x columns are the scores moving operand. Key order is free (softmax
  sums over keys), so the host rolls x^T per core to put the core's query
  slice at columns 0:SQ - no separate query buffer or DMA.

Per-core dataflow (mode-dependent):
  k''^T = gT.T @ x^T                  (PE, bf16, N=512 x8)
  [V | alpha] chunks = x^T-chunk.T @ wv_aug   (PE, bf16, N=130 x32)
  scoresT[t-chunk, s] = k''-chunk.T @ x^T[:, queries]   (PE bf16 -> PSUM f32)
  et = exp(scores + alpha - SHIFT)    (ACT, PSUM -> SBUF)
  mode "bf16": et bf16; AV bf16 matmuls; denominator chunk-accumulated on DVE
    (bf16 2x) folded by one ones-matmul per pass.
  mode "fp8": et fp8e4 written into pair tiles [128, 2*sw]; AV and denominator
    are fp8 DoubleRow matmuls (2 key-chunks per matmul, 0.5 cyc/row);
    denominator costs no DVE time. Optionally (noff>0) some chunks' exp is
    computed on DVE via the Schraudolph bit-trick (y*2^23/ln2 + C written as
    int32, bitcast to f32) + a Pool/DVE fp8 convert, offloading the ACT.
  out^T = AV / denom + bv  (DVE), DMA'd out as [e, s]; host flips layout.
"""

import sys

import numpy as np

for _p in ("/opt/trn_rl_repo", "/opt/pypackages"):
    if _p not in sys.path:
        sys.path.append(_p)

import ml_dtypes

B, S, D = 4, 4096, 128
N_CORES = 8
SQ = S // 2  # queries per core
SHIFT = 3.5  # subtracted inside exp; cancels in softmax, keeps et fp8-safe

# Schraudolph exp constants: exp(y) ~= bitcast_f32(int32(y*K1 + K2))
K1 = float(2 ** 23 / np.log(2.0))
K2 = float((127 << 23) - 366393.0)

# mode: "bf16" | "fp8"; noff: chunks per pass whose exp runs on DVE(+Pool)
MODE = "bf16"
NOFF = 0
V_RESID = False  # extra fp8 residual matmul for V (halves V quantization err)
NOFF_POOL = True  # fp8 convert of offloaded chunks on GPSIMD (else DVE)


def build_attention_bass(s=S, sq=SQ, sw=1024, mode=MODE, noff=NOFF,
                         v_resid=V_RESID, noff_pool=NOFF_POOL):
    """Build the single-core SPMD Bass program."""
    import concourse.mybir as mybir
    import concourse.tile as tile
    from concourse import bacc
    from contextlib import ExitStack

    f32 = mybir.dt.float32
    bf16 = mybir.dt.bfloat16
    fp8 = mybir.dt.float8e4
    i32 = mybir.dt.int32
    FT = mybir.ActivationFunctionType
    ALU = mybir.AluOpType
    DR = mybir.MatmulPerfMode.DoubleRow

    tch = s // 128          # key chunks
    n_pass = sq // sw       # query passes
    nw = min(512, sw)       # matmul N width (PSUM bank limit for f32 out)
    jn = sw // nw           # matmuls per pass-width
    nk = s // 512           # k'' projection matmuls
    assert tch % 2 == 0 and sq % sw == 0 and sw % nw == 0

    # which chunks' exp is offloaded to DVE(+Pool): last chunk of each of
    # noff equal buckets (uniform spread, skips chunk 0 when noff < tch)
    off_set = {c for c in range(tch)
               if (c * noff) // tch != ((c + 1) * noff) // tch}

    nc = bacc.Bacc("TRN2", target_bir_lowering=False, debug=False)

    xT = nc.dram_tensor("xT", [D, s], bf16, kind="ExternalInput").ap()
    gT = nc.dram_tensor("gT", [D, D], bf16, kind="ExternalInput").ap()
    wvT = nc.dram_tensor("wvT", [D, D + 2], bf16, kind="ExternalInput").ap()
    bv = nc.dram_tensor("bv", [D, 1], f32, kind="ExternalInput").ap()
    out_d = nc.dram_tensor("out", [D, sq], f32, kind="ExternalOutput").ap()

    with tile.TileContext(nc) as tc, ExitStack() as ctx:
        const = ctx.enter_context(tc.tile_pool(name="const", bufs=1))
        big = ctx.enter_context(tc.tile_pool(name="big", bufs=1))
        exp_pool = ctx.enter_context(tc.tile_pool(name="exp", bufs=4))
        epi = ctx.enter_context(tc.tile_pool(name="epi", bufs=2))
        outp = ctx.enter_context(tc.tile_pool(name="outp", bufs=3))
        if noff > 0:
            ei_pool = ctx.enter_context(tc.tile_pool(name="ei", bufs=2))

        g_sb = const.tile([D, D], bf16, tag="g")
        wv_sb = const.tile([D, D + 2], bf16, tag="wv")
        bv_sb = const.tile([D, 1], f32, tag="bv")

        xT_sb = big.tile([D, s], bf16, tag="xT")
        # input DMAs: weights first, then x^T in 512-col chunks (cols 0:1024
        # serve both the first k'' chunks and the pass-0 queries)
        nc.sync.dma_start(g_sb[:], gT)
        nc.sync.dma_start(wv_sb[:], wvT)
        for st in range(0, s, 512):
            nc.sync.dma_start(xT_sb[:, st:st + 512], xT[:, st:st + 512])
        nc.sync.dma_start(bv_sb[:], bv)

        kt_sb = big.tile([D, s], bf16, tag="kt")
        alpha_sb = big.tile([128, tch], f32, tag="alpha")
        if mode == "fp8":
            v8_sb = big.tile([128, 128 * tch], fp8, tag="v8")
            ones8 = const.tile([128, 256], fp8, tag="ones8")
            nc.vector.memset(ones8[:], 1.0)
            if v_resid:
                vr8_sb = big.tile([128, 128 * tch], fp8, tag="vr8")
        else:
            v_sb = big.tile([128, 128 * tch], bf16, tag="v")
            ones_sb = const.tile([128, 128], bf16, tag="ones")
            nc.vector.memset(ones_sb[:], 1.0)
            accdv = ctx.enter_context(tc.tile_pool(name="accdv", bufs=2))
        if noff > 0:
            alpha2_sb = big.tile([128, tch], f32, tag="alpha2")

        with tc.tile_pool(name="scps", bufs=2, space="PSUM") as scps:
            # projection pools live only until the pass loop starts
            qkv_ctx = ExitStack()
            kps = qkv_ctx.enter_context(
                tc.tile_pool(name="kps", bufs=2, space="PSUM"))
            vps = qkv_ctx.enter_context(
                tc.tile_pool(name="vps", bufs=2, space="PSUM"))

            def emit_k(j):
                st = j * 512
                kp = kps.tile([128, 512], f32, tag="kp")
                nc.tensor.matmul(kp[:], g_sb[:], xT_sb[:, st:st + 512])
                nc.vector.tensor_copy(kt_sb[:, st:st + 512], kp[:])

            def emit_v_alpha(c):
                vp = vps.tile([128, D + 2], f32, tag="vp")
                xc = xT_sb[:, c * 128:(c + 1) * 128]
                nc.tensor.matmul(vp[:], xc, wv_sb[:])
                # alpha column with the exp shift folded in
                nc.vector.tensor_scalar_add(
                    alpha_sb[:, c:c + 1], vp[:, D:D + 1], -SHIFT)
                dst = slice(c * 128, (c + 1) * 128)
                if mode == "fp8":
                    nc.vector.tensor_copy(v8_sb[:, dst], vp[:, :D])
                    if v_resid:
                        nc.vector.tensor_tensor(
                            vr8_sb[:, dst], vp[:, :D], v8_sb[:, dst],
                            ALU.subtract)
                else:
                    nc.vector.tensor_copy(v_sb[:, dst], vp[:, :D])

            pair_tiles = {}

            def emit_scores_exp(p, c):
                """scores chunk c of pass p -> PSUM; exp -> et (half-)tile."""
                sc = scps.tile([128, sw], f32, tag="sc")
                kc = kt_sb[:, c * 128:(c + 1) * 128]
                for j in range(jn):
                    nc.tensor.matmul(
                        sc[:, j * nw:(j + 1) * nw], kc,
                        xT_sb[:, p * sw + j * nw: p * sw + (j + 1) * nw])
                abias = alpha_sb[:, c:c + 1]
                if mode == "fp8":
                    if c % 2 == 0:
                        pair_tiles[(p, c // 2)] = exp_pool.tile(
                            [128, 2 * sw], fp8, tag="et", name="et_pair")
                    et = pair_tiles[(p, c // 2)]
                    half = et[:, (c % 2) * sw:(c % 2 + 1) * sw]
                    if c in off_set:
                        ei = ei_pool.tile([128, sw], i32, tag="ei")
                        nc.vector.tensor_scalar(
                            ei[:], sc[:], K1, alpha2_sb[:, c:c + 1],
                            ALU.mult, ALU.add)
                        eng = nc.gpsimd if noff_pool else nc.vector
                        eng.tensor_copy(half, ei[:].bitcast(f32))
                    else:
                        nc.scalar.activation(half, sc[:], FT.Exp,
                                             bias=abias, scale=1.0)
                    return None
                et = exp_pool.tile([128, sw], bf16, tag="et")
                nc.scalar.activation(et[:], sc[:], FT.Exp,
                                     bias=abias, scale=1.0)
                return et

            def epilogue(p, acc_o, acc_d):
                # normalize + bias, DMA the [e, s] block out; finer blocks on
                # the last pass so tail DMAs start earlier
                bw = 256 if p == n_pass - 1 else nw
                for b0 in range(0, sw, bw):
                    recip = epi.tile([128, bw], f32, tag="recip")
                    nc.vector.reciprocal_approx_fast(
                        recip[:], acc_d[:, b0:b0 + bw])
                    norm = epi.tile([128, bw], f32, tag="norm")
                    nc.vector.tensor_mul(norm[:], acc_o[:, b0:b0 + bw],
                                         recip[:])
                    norm2 = outp.tile([128, bw], f32, tag="norm2")
                    nc.vector.tensor_scalar_add(norm2[:], norm[:], bv_sb[:])
                    c0 = p * sw + b0
                    nc.sync.dma_start(out_d[:, c0:c0 + bw], norm2[:])

            # ---- projections: enough for the pass-0 pipeline first
            nk_pre = min(2, nk)
            nv_pre = min(4, tch)
            for j in range(nk_pre):
                emit_k(j)
            for c in range(nv_pre):
                emit_v_alpha(c)
            if noff > 0:
                # alpha2 = alpha*K1 + K2 (Schraudolph pre-bias, shift incl.)
                nc.vector.tensor_scalar(
                    alpha2_sb[:, :nv_pre], alpha_sb[:, :nv_pre], K1, K2,
                    ALU.mult, ALU.add)

            sched = [(pp, cc) for pp in range(n_pass) for cc in range(tch)]
            npre = min(4, len(sched))  # chunks of score/exp pre-emitted
            pre = []
            for i in range(npre):
                r = emit_scores_exp(*sched[i])
                if mode != "fp8":
                    pre.append(r)
            cursor = [npre]

            # rest of the projections (fills PE while ACT runs first exps)
            for j in range(nk_pre, nk):
                emit_k(j)
            for c in range(nv_pre, tch):
                emit_v_alpha(c)
            if noff > 0 and nv_pre < tch:
                nc.vector.tensor_scalar(
                    alpha2_sb[:, nv_pre:], alpha_sb[:, nv_pre:], K1, K2,
                    ALU.mult, ALU.add)

            qkv_ctx.close()
            acc_ctx = ExitStack()
            accps = acc_ctx.enter_context(
                tc.tile_pool(name="accps", bufs=1, space="PSUM"))

            def emit_upto(idx):
                while cursor[0] < len(sched) and cursor[0] <= idx:
                    pp, cc = sched[cursor[0]]
                    cursor[0] += 1
                    r = emit_scores_exp(pp, cc)
                    if mode != "fp8":
                        pre.append(r)

            # ---- attention passes
            if mode == "fp8":
                ones_ap = ones8[:].rearrange("p (two m) -> p two m", two=2)
                n_pair = tch // 2
                for p in range(n_pass):
                    acc_o = accps.tile([128, sw], f32, tag="acco")
                    acc_d = accps.tile([128, sw], f32, tag="accd")
                    for jp in range(n_pair):
                        # keep the score/exp pipeline ~4 chunks ahead
                        emit_upto(p * tch + jp * 2 + 5)
                        et = pair_tiles.pop((p, jp))
                        et3 = et[:].rearrange("p (two n) -> p two n", two=2)
                        v_ap = v8_sb[:, jp * 256:(jp + 1) * 256].rearrange(
                            "p (two m) -> p two m", two=2)
                        first = jp == 0
                        lastd = jp == n_pair - 1
                        lasto = lastd and not v_resid
                        for j in range(jn):
                            rhs = et3[:, :, j * nw:(j + 1) * nw]
                            ob = acc_o[:, j * nw:(j + 1) * nw]
                            nc.tensor.matmul(ob, v_ap, rhs, start=first,
                                             stop=lasto, perf_mode=DR)
                            nc.tensor.matmul(
                                acc_d[:, j * nw:(j + 1) * nw], ones_ap, rhs,
                                start=first, stop=lastd, perf_mode=DR)
                            if v_resid:
                                vr_ap = vr8_sb[
                                    :, jp * 256:(jp + 1) * 256].rearrange(
                                    "p (two m) -> p two m", two=2)
                                nc.tensor.matmul(ob, vr_ap, rhs, start=False,
                                                 stop=lastd, perf_mode=DR)
                    epilogue(p, acc_o, acc_d)
            else:
                for p in range(n_pass):
                    acc_o = accps.tile([128, sw], f32, tag="acco")
                    acc_d = accps.tile([128, sw], f32, tag="accd")
                    acc_dv = accdv.tile([128, sw], bf16, tag="accdv")
                    for c in range(tch):
                        et = pre.pop(0)
                        emit_upto(p * tch + c + 2)
                        vc = v_sb[:, c * 128:(c + 1) * 128]
                        for j in range(jn):
                            nc.tensor.matmul(
                                acc_o[:, j * nw:(j + 1) * nw], vc,
                                et[:, j * nw:(j + 1) * nw],
                                start=(c == 0), stop=(c == tch - 1))
                        if c == 0:
                            nc.vector.tensor_copy(acc_dv[:], et[:])
                        else:
                            nc.vector.tensor_add(acc_dv[:], acc_dv[:], et[:])
                    for j in range(jn):
                        nc.tensor.matmul(acc_d[:, j * nw:(j + 1) * nw],
                                         ones_sb[:],
                                         acc_dv[:, j * nw:(j + 1) * nw],
                                         start=True, stop=True)
                    epilogue(p, acc_o, acc_d)
            acc_ctx.close()
    nc.compile()
    return nc


def make_in_maps(x, Wq, bq, Wk, Wv, bv, s=S, sq=SQ, n_cores=N_CORES):
    """Per-core input dict list. Core c -> batch c//(cores per batch);
    x^T is rolled so the core's query slice sits at columns 0:sq."""
    bf = ml_dtypes.bfloat16
    x = np.asarray(x, np.float64)
    nb = x.shape[0]
    per_b = n_cores // nb
    d = x.shape[2]
    g_t = (np.asarray(Wk, np.float64).T @ np.asarray(Wq, np.float64)
           / np.sqrt(d))
    wtl = (np.asarray(Wk, np.float64).T @ np.asarray(bq, np.float64)
           / np.sqrt(d)).reshape(d, 1)
    wv_t = np.asarray(Wv, np.float64).T
    wv_aug = np.concatenate([wv_t, wtl, wtl], axis=1)
    bvc = np.asarray(bv, np.float32).reshape(d, 1)
    maps = []
    for c in range(n_cores):
        b, h = c // per_b, c % per_b
        xt = x[b].T  # [d, s]
        roll = np.concatenate([xt[:, h * sq:], xt[:, :h * sq]], axis=1)
        maps.append({
            "xT": np.ascontiguousarray(roll.astype(bf)),
            "gT": np.ascontiguousarray(g_t.astype(bf)),
            "wvT": np.ascontiguousarray(wv_aug.astype(bf)),
            "bv": np.ascontiguousarray(bvc),
        })
    return maps


_NC_CACHE = {}


def _get_nc():
    if "nc" not in _NC_CACHE:
        _NC_CACHE["nc"] = build_attention_bass()
    return _NC_CACHE["nc"]


def run_on_hw(inputs, trace=False, **kw):
    from concourse.bass_utils import run_bass_kernel_spmd
    nc = _get_nc()
    maps = make_in_maps(inputs["x"], inputs["Wq"], inputs["bq"], inputs["Wk"],
                        inputs["Wv"], inputs["bv"])
    res = run_bass_kernel_spmd(nc, maps, core_ids=list(range(N_CORES)),
                               trace=trace, **kw)
    nb = np.asarray(inputs["x"]).shape[0]
    per_b = N_CORES // nb
    out = np.empty((nb, S * D), np.float32)
    for c in range(N_CORES):
        b, h = c // per_b, c % per_b
        # device returns out^T [D, SQ]; final layout flip happens here
        out[b, h * SQ * D:(h + 1) * SQ * D] = \
            np.asarray(res.results[c]["out"]).T.reshape(-1)
    return out, res


def kernel(**inputs):
    out, _ = run_on_hw(inputs, trace=False)
    return out
